# revision 5
# baseline (speedup 1.0000x reference)
"""Pairwise cosine-similarity (normalize -> x @ x.T) + Linear(1,2) affine, on 8 trn2 cores.

Strategy (data-parallel over rows of x, per sharding hint):
  - Each core owns a 512-row slice of the 4096x4096 similarity matrix.
  - Per core: load full x [4096,768] fp32, compute row norms in fp32
    (square+row-sum -> clamp -> rsqrt), scale rows by 1/norm and cast to
    fp16, transpose via the PE (128x128 tiles) into xnT [768, 4096] fp16.
  - sim tile [128,512] = sum_k xnT_k[:, own cols].T @ xnT_k[:, n cols]
    (fp16 matmul, fp32 PSUM accumulation; 1 cyc/row vs 4 for fp32).
  - Epilogue: out[...,k] = sim * w_k + b_k with immediate scalars
    (ACT does k=0, DVE does k=1), interleaved [128, 512, 2] fp32 in SBUF,
    contiguous DMA to the output slice.
"""

import numpy as np
from contextlib import ExitStack

import concourse.bass as bass
import concourse.tile as tile
from concourse import mybir
from concourse.bass_utils import run_bass_kernel_spmd

B, D, NCORES = 4096, 768, 8
BC = B // NCORES          # 512 rows per core
P = 128                   # partitions
KT = D // P               # 6 contraction tiles
NT = 512                  # sim column tile (one PSUM bank of fp32)
F16 = mybir.dt.float16
F32 = mybir.dt.float32
AF = mybir.ActivationFunctionType
ALU = mybir.AluOpType

LAST_RESULTS = None       # test harness peeks at exec_time_ns here


def _legalize_single_wait(bir_bytes: bytes) -> bytes:
    """This container's walrus accepts at most ONE sync wait per instruction,
    while Tile attaches several. Split extras into standalone EventSemaphore
    instructions inserted just before the owner (same engine stream, so the
    sequencer stalls at the same program point; schedule order is a global
    topological order, so earlier stalls cannot deadlock)."""
    import json

    d = json.loads(bir_bytes)
    n_split = 0
    for f in d.get("functions", []):
        for bb in f.get("blocks", []):
            insts = bb.get("instructions", [])
            out = []
            for ins in insts:
                si = ins.get("sync_info") or {}
                waits = si.get("on_wait") or []
                if len(waits) > 1:
                    keep = waits[-1]
                    for i, w in enumerate(waits[:-1]):
                        n_split += 1
                        out.append({
                            "debug": ins.get("debug", 0),
                            "engine": ins["engine"],
                            "ins": [],
                            "name": f"{ins['name']}__w{i}",
                            "opcode": "EventSemaphore",
                            "outs": [],
                            "sync_info": {"on_update": [], "on_wait": [w]},
                        })
                    si["on_wait"] = [keep]
                out.append(ins)
            bb["instructions"] = out
    return json.dumps(d).encode()


def _install_walrus_shim():
    """Route every BIR->NEFF compile through the single-wait legalizer."""
    import concourse.bass2jax as b2j
    import concourse.bass_utils as bu

    if getattr(bu, "_single_wait_shim", False):
        return
    orig = bu.compile_bir_kernel

    def patched(bir_json: bytes, tmpdir, neff_name: str = "file.neff"):
        return orig(_legalize_single_wait(bir_json), tmpdir, neff_name)

    bu.compile_bir_kernel = patched
    b2j.compile_bir_kernel = patched
    bu._single_wait_shim = True


_install_walrus_shim()


def _build(w0: float, w1: float, b0: float, b1: float) -> bass.Bass:
    nc = bass.Bass("TRN2", target_bir_lowering=False, debug=False, num_devices=NCORES)
    x = nc.dram_tensor("x", [B, D], F32, kind="ExternalInput").ap()
    xr = nc.dram_tensor("xrows", [BC, D], F32, kind="ExternalInput").ap()
    out = nc.dram_tensor("out", [BC, B, 2], F32, kind="ExternalOutput").ap()
    ident_d = nc.inline_tensor(np.eye(P, dtype=np.float16), "ident")

    with tile.TileContext(nc) as tc, ExitStack() as ctx:
        xpool = ctx.enter_context(tc.tile_pool(name="xin", bufs=4))
        sqpool = ctx.enter_context(tc.tile_pool(name="sq", bufs=2))
        stat = ctx.enter_context(tc.tile_pool(name="stat", bufs=4))
        fpool = ctx.enter_context(tc.tile_pool(name="xn16", bufs=4))
        tpsum = ctx.enter_context(tc.tile_pool(name="tpsum", bufs=4, space="PSUM"))
        spsum = ctx.enter_context(tc.tile_pool(name="spsum", bufs=3, space="PSUM"))
        opool = ctx.enter_context(tc.tile_pool(name="outt", bufs=4))
        big = ctx.enter_context(tc.tile_pool(name="big", bufs=1))

        ident = big.tile([P, P], F16, name="ident_sb")
        nc.sync.dma_start(ident, ident_d.ap())
        xnT = big.tile([P, KT, B], F16, name="xnT")     # normalized x, transposed
        ownT = big.tile([P, KT, BC], F16, name="ownT")  # same for this core's rows

        def prep(src_ap, t, dst, alt):
            """Row-tile t of src: norms -> scale+cast fp16 -> transpose into dst."""
            xt = xpool.tile([P, D], F32, tag="xt", name=f"xt{t}")
            nc.sync.dma_start(xt, src_ap[t * P:(t + 1) * P, :])
            ss = stat.tile([P, 1], F32, tag="ss", name=f"ss{t}")
            sq = sqpool.tile([P, D], F32, tag="sq", name=f"sqt{t}")
            nc.vector.scalar_tensor_tensor(
                sq, xt, 1.0, xt,
                op0=ALU.bypass, op1=ALU.mult, accum_out=ss,
            )
            ss2 = stat.tile([P, 1], F32, tag="ss2", name=f"ss2{t}")
            nc.vector.tensor_scalar_max(ss2, ss, 1e-16)  # max(||x||, eps) guard
            rin = stat.tile([P, 1], F32, tag="rin", name=f"rin{t}")
            nc.vector.reciprocal(rin, ss2)
            r = stat.tile([P, 1], F32, tag="r", name=f"r{t}")
            nc.scalar.sqrt(r, rin)                       # rsqrt(sumsq)
            xn = fpool.tile([P, D], F16, tag="xn", name=f"xn{t}")
            nc.vector.tensor_scalar_mul(xn, xt, r)       # normalize + cast fp16
            for k in range(KT):
                pt = tpsum.tile([P, P], F16, tag="pt", name=f"pt{t}_{k}")
                nc.tensor.transpose(pt, xn[:, k * P:(k + 1) * P], ident)
                d = dst[:, k, t * P:(t + 1) * P]
                if (t * KT + k) % 2 == 0:
                    nc.vector.tensor_copy(d, pt)
                else:
                    nc.scalar.copy(d, pt)

        for t in range(BC // P):            # own rows first: unblocks matmuls
            prep(xr, t, ownT, alt=bool(t % 2))
        for t in range(B // P):
            prep(x, t, xnT, alt=bool(t % 2))

        for n in range(B // NT):
            for m in range(BC // P):
                ps = spsum.tile([P, NT], F32, tag="ps", name=f"ps{n}_{m}")
                for k in range(KT):
                    nc.tensor.matmul(
                        ps,
                        ownT[:, k, m * P:(m + 1) * P],
                        xnT[:, k, n * NT:(n + 1) * NT],
                        start=(k == 0), stop=(k == KT - 1),
                    )
                ot = opool.tile([P, NT, 2], F32, tag="ot", name=f"ot{n}_{m}")
                nc.scalar.activation(ot[:, :, 0:1], ps, AF.Copy, bias=b0, scale=w0)
                nc.vector.tensor_scalar(
                    ot[:, :, 1:2], ps, w1, b1, op0=ALU.mult, op1=ALU.add
                )
                nc.sync.dma_start(out[m * P:(m + 1) * P, n * NT:(n + 1) * NT, :], ot)
    return nc


def kernel(x, fc_w, fc_b):
    global LAST_RESULTS
    x = np.ascontiguousarray(np.asarray(x, dtype=np.float32))
    fc_w = np.asarray(fc_w, dtype=np.float32)
    fc_b = np.asarray(fc_b, dtype=np.float32)
    nc = _build(float(fc_w[0, 0]), float(fc_w[1, 0]),
                float(fc_b[0]), float(fc_b[1]))
    in_maps = [
        {"x": x, "xrows": np.ascontiguousarray(x[c * BC:(c + 1) * BC])}
        for c in range(NCORES)
    ]
    res = run_bass_kernel_spmd(nc, in_maps, core_ids=list(range(NCORES)))
    LAST_RESULTS = res
    return np.concatenate([res.results[c]["out"] for c in range(NCORES)], axis=0)


# revision 7
# speedup vs baseline: 1.0795x; 1.0795x over previous
"""Pairwise cosine-similarity (normalize -> x @ x.T) + Linear(1,2) affine, on 8 trn2 cores.

Strategy (data-parallel over rows of x, per sharding hint):
  - Each core owns a 512-row slice of the 4096x4096 similarity matrix.
  - Per core: load full x [4096,768] fp32, compute row norms in fp32
    (square+row-sum -> clamp -> rsqrt), scale rows by 1/norm and cast to
    fp16, transpose via the PE (128x128 tiles) into xnT [768, 4096] fp16.
  - sim tile [128,512] = sum_k xnT_k[:, own cols].T @ xnT_k[:, n cols]
    (fp16 matmul, fp32 PSUM accumulation; 1 cyc/row vs 4 for fp32).
  - Epilogue: out[...,k] = sim * w_k + b_k with immediate scalars
    (ACT does k=0, DVE does k=1), interleaved [128, 512, 2] fp32 in SBUF,
    contiguous DMA to the output slice.
"""

import numpy as np
from contextlib import ExitStack

import concourse.bass as bass
import concourse.tile as tile
from concourse import mybir
from concourse.bass_utils import run_bass_kernel_spmd

B, D, NCORES = 4096, 768, 8
BC = B // NCORES          # 512 rows per core
P = 128                   # partitions
KT = D // P               # 6 contraction tiles
NT = 512                  # sim column tile (one PSUM bank of fp32)
F16 = mybir.dt.float16
F32 = mybir.dt.float32
AF = mybir.ActivationFunctionType
ALU = mybir.AluOpType

LAST_RESULTS = None       # test harness peeks at exec_time_ns here


def _legalize_single_wait(bir_bytes: bytes) -> bytes:
    """This container's walrus accepts at most ONE sync wait per instruction,
    while Tile attaches several. Split extras into standalone EventSemaphore
    instructions inserted just before the owner (same engine stream, so the
    sequencer stalls at the same program point; schedule order is a global
    topological order, so earlier stalls cannot deadlock)."""
    import json

    d = json.loads(bir_bytes)
    n_split = 0
    for f in d.get("functions", []):
        for bb in f.get("blocks", []):
            insts = bb.get("instructions", [])
            out = []
            for ins in insts:
                si = ins.get("sync_info") or {}
                waits = si.get("on_wait") or []
                if len(waits) > 1:
                    keep = waits[-1]
                    for i, w in enumerate(waits[:-1]):
                        n_split += 1
                        out.append({
                            "debug": ins.get("debug", 0),
                            "engine": ins["engine"],
                            "ins": [],
                            "name": f"{ins['name']}__w{i}",
                            "opcode": "EventSemaphore",
                            "outs": [],
                            "sync_info": {"on_update": [], "on_wait": [w]},
                        })
                    si["on_wait"] = [keep]
                out.append(ins)
            bb["instructions"] = out
    return json.dumps(d).encode()


def _install_walrus_shim():
    """Route every BIR->NEFF compile through the single-wait legalizer."""
    import concourse.bass2jax as b2j
    import concourse.bass_utils as bu

    if getattr(bu, "_single_wait_shim", False):
        return
    orig = bu.compile_bir_kernel

    def patched(bir_json: bytes, tmpdir, neff_name: str = "file.neff"):
        return orig(_legalize_single_wait(bir_json), tmpdir, neff_name)

    bu.compile_bir_kernel = patched
    b2j.compile_bir_kernel = patched
    bu._single_wait_shim = True


_install_walrus_shim()


def _build(w0: float, w1: float, b0: float, b1: float) -> bass.Bass:
    nc = bass.Bass("TRN2", target_bir_lowering=False, debug=False, num_devices=NCORES)
    x = nc.dram_tensor("x", [B, D], F32, kind="ExternalInput").ap()
    xr = nc.dram_tensor("xrows", [BC, D], F32, kind="ExternalInput").ap()
    out = nc.dram_tensor("out", [BC, B, 2], F32, kind="ExternalOutput").ap()
    ident_d = nc.inline_tensor(np.eye(P, dtype=np.float16), "ident")

    with tile.TileContext(nc) as tc, ExitStack() as ctx:
        xpool = ctx.enter_context(tc.tile_pool(name="xin", bufs=8))
        sqpool = ctx.enter_context(tc.tile_pool(name="sq", bufs=3))
        stat = ctx.enter_context(tc.tile_pool(name="stat", bufs=6))
        fpool = ctx.enter_context(tc.tile_pool(name="xn16", bufs=6))
        tpsum = ctx.enter_context(tc.tile_pool(name="tpsum", bufs=4, space="PSUM"))
        spsum = ctx.enter_context(tc.tile_pool(name="spsum", bufs=3, space="PSUM"))
        opool = ctx.enter_context(tc.tile_pool(name="outt", bufs=4))
        big = ctx.enter_context(tc.tile_pool(name="big", bufs=1))

        ident = big.tile([P, P], F16, name="ident_sb")
        nc.sync.dma_start(ident, ident_d.ap())
        xnT = big.tile([P, KT, B], F16, name="xnT")     # normalized x, transposed
        ownT = big.tile([P, KT, BC], F16, name="ownT")  # same for this core's rows

        def prep(src_ap, t, dst, pfx, eng):
            """Row-tile t of src: norms -> scale+cast fp16 -> transpose into
            dst[:, :, t*P:(t+1)*P].  eng rotates the square+rowsum pass across
            engines to balance load."""
            xt = xpool.tile([P, D], F32, tag="xt", name=f"xt{pfx}{t}")
            nc.sync.dma_start(xt, src_ap[t * P:(t + 1) * P, :])
            ss = stat.tile([P, 1], F32, tag="ss", name=f"ss{pfx}{t}")
            sq = sqpool.tile([P, D], F16, tag="sq", name=f"sqt{pfx}{t}")
            if eng == 0:
                nc.scalar.activation(sq, xt, AF.Square, accum_out=ss)
            else:
                nc.vector.scalar_tensor_tensor(
                    sq, xt, 1.0, xt,
                    op0=ALU.bypass, op1=ALU.mult, accum_out=ss,
                )
            ss2 = stat.tile([P, 1], F32, tag="ss2", name=f"ss2{pfx}{t}")
            nc.vector.tensor_scalar_max(ss2, ss, 1e-16)  # max(||x||, eps) guard
            rin = stat.tile([P, 1], F32, tag="rin", name=f"rin{pfx}{t}")
            nc.vector.reciprocal(rin, ss2)
            r = stat.tile([P, 1], F32, tag="r", name=f"r{pfx}{t}")
            nc.scalar.sqrt(r, rin)                       # rsqrt(sumsq)
            xn = fpool.tile([P, D], F16, tag="xn", name=f"xn{pfx}{t}")
            if eng == 0:
                nc.vector.tensor_scalar_mul(xn, xt, r)   # normalize + cast fp16
            else:
                nc.scalar.activation(xn, xt, AF.Copy, scale=r)
            for k in range(KT):
                pt = tpsum.tile([P, P], F16, tag="pt", name=f"pt{pfx}{t}_{k}")
                nc.tensor.transpose(pt, xn[:, k * P:(k + 1) * P], ident)
                dd = dst[:, k, t * P:(t + 1) * P]
                if (t * KT + k) % 2 == 0:
                    nc.vector.tensor_copy(dd, pt)
                else:
                    nc.scalar.copy(dd, pt)

        TPB = NT // P                       # 4 row-tiles per n-block
        for t in range(BC // P):            # own rows first: unblocks matmuls
            prep(xr, t, ownT, "o", eng=t % 2)

        for n in range(B // NT):            # pipelined n-blocks
            for j in range(TPB):            # prep the 4 row-tiles this block needs
                t = n * TPB + j
                prep(x, t, xnT, "x", eng=t % 2)
            for m in range(BC // P):
                ps = spsum.tile([P, NT], F32, tag="ps", name=f"ps{n}_{m}")
                for k in range(KT):
                    nc.tensor.matmul(
                        ps,
                        ownT[:, k, m * P:(m + 1) * P],
                        xnT[:, k, n * NT:(n + 1) * NT],
                        start=(k == 0), stop=(k == KT - 1),
                    )
                ot = opool.tile([P, NT, 2], F32, tag="ot", name=f"ot{n}_{m}")
                nc.scalar.activation(ot[:, :, 0:1], ps, AF.Copy, bias=b0, scale=w0)
                nc.vector.tensor_scalar(
                    ot[:, :, 1:2], ps, w1, b1, op0=ALU.mult, op1=ALU.add
                )
                nc.sync.dma_start(out[m * P:(m + 1) * P, n * NT:(n + 1) * NT, :], ot)
    return nc


def kernel(x, fc_w, fc_b):
    global LAST_RESULTS
    x = np.ascontiguousarray(np.asarray(x, dtype=np.float32))
    fc_w = np.asarray(fc_w, dtype=np.float32)
    fc_b = np.asarray(fc_b, dtype=np.float32)
    nc = _build(float(fc_w[0, 0]), float(fc_w[1, 0]),
                float(fc_b[0]), float(fc_b[1]))
    in_maps = [
        {"x": x, "xrows": np.ascontiguousarray(x[c * BC:(c + 1) * BC])}
        for c in range(NCORES)
    ]
    res = run_bass_kernel_spmd(nc, in_maps, core_ids=list(range(NCORES)))
    LAST_RESULTS = res
    return np.concatenate([res.results[c]["out"] for c in range(NCORES)], axis=0)
